# revision 12
# baseline (speedup 1.0000x reference)
"""Trainium2 Bass kernel for nn_Attention_47682726920277.

Causal multi-head attention with RoPE:
  q/k/v = x @ w{q,k,v}.T ; RoPE(q, k) ; att = softmax(mask(q k^T / 8)) ; out = (att v) @ wo.T
Shapes: x [2, 2048, 1024], 16 heads of dim 64, fp32.

Sharding (8 cores): data-parallel over batch (2) x tensor-parallel over heads (4 per
core). Each core computes its 4 heads' attention and a partial out via its wo row
block; the final all-reduce is the host-side sum of the 4 partials per batch.

Per-core design (v2 — software-pipelined):
  - Q,K produced transposed (QT/KT [256ch, T] fp16) so scores come out as S^T [k, q];
    V is augmented with a ones column so row 64 of the PV accumulator is the softmax
    denominator.  Exp runs on the scalar engine with the 1/8 scale fused; the scalar
    engine runs *only* exp (one activation table, loaded once).
  - RoPE runs at the DVE 2x fp16 rate: one cast from PSUM, a partition-XOR-32 swap
    done by 4 small SBUF->SBUF DMAs, then 3 full-width fp16 multiplies/adds.
  - Normalization: DVE reciprocal of the sums row, gpsimd partition-broadcast, one
    DVE multiply into attnT.  No DRAM roundtrip, no Ln on the scalar engine.
  - PSUM drains via direct DMA (PSUM->SBUF staging, PSUM->DRAM for the wo output)
    instead of vector-engine copies.
  - Emission interleaves the two head-pairs' attention per q-chunk and feeds the
    tensor engine projection/wo matmuls as fillers *inside* the attention kb-loop,
    so the PE never idles waiting on exp and the HAM clock gate stays at 2.4 GHz.
"""
import sys
import types
import numpy as np

B = 2
T = 2048
D = 1024
H = 16
HD = 64
NCORES = 8
GROUPS = NCORES // B          # head-groups per batch
HPC = H // GROUPS             # heads per core = 4
CH = HPC * HD                 # channels per core = 256
NQ = 512                      # PSUM bank width (fp32)
P = 128

_prog_cache = {}


def _install_ntff_shim():
    """The agent image's antenv lacks axon_hooks; inject it so trace=True works."""
    try:
        import antenv.axon_hooks  # noqa: F401
        return
    except ImportError:
        pass
    try:
        import trn_agent_boot.trn_boot as tb
        hook = tb._ntff_profile_via_ctypes('/opt/axon/libaxon_pjrt.so')
        if hook is None:
            return
        mod = types.ModuleType('antenv.axon_hooks')
        mod.get_axon_ntff_profile_hook = lambda: hook
        mod.set_axon_ntff_profile_hook = lambda h: None
        sys.modules['antenv.axon_hooks'] = mod
        import antenv
        antenv.axon_hooks = mod
    except Exception:
        pass


def _build_program(causal: bool):
    import concourse.bass as bass
    from concourse import bacc
    import concourse.tile as tile
    from concourse import mybir

    F32 = mybir.dt.float32
    F16 = mybir.dt.float16
    AF = mybir.ActivationFunctionType
    MUL = mybir.AluOpType.mult
    ADD = mybir.AluOpType.add

    NT = T // NQ          # proj/attention q-chunks (4)
    NKB = T // P          # k-blocks (16)
    DB = D // P           # d-blocks (8)
    CB = CH // P          # channel blocks = head-pair blocks (2)

    nc = bacc.Bacc("TRN2", target_bir_lowering=False, debug=False)

    xT = nc.dram_tensor("xT", [D, T], F16, kind="ExternalInput").ap()
    wqT = nc.dram_tensor("wqT", [D, CH], F16, kind="ExternalInput").ap()
    wkT = nc.dram_tensor("wkT", [D, CH], F16, kind="ExternalInput").ap()
    wvT = nc.dram_tensor("wvT", [D, CH], F16, kind="ExternalInput").ap()
    woT = nc.dram_tensor("woT", [CH, D], F16, kind="ExternalInput").ap()
    cosS = nc.dram_tensor("cosS", [P, T], F16, kind="ExternalInput").ap()
    sinS = nc.dram_tensor("sinS", [P, T], F16, kind="ExternalInput").ap()
    ident = nc.dram_tensor("ident", [P, P], F16, kind="ExternalInput").ap()
    triB = nc.dram_tensor("triB", [P, P], F16, kind="ExternalInput").ap()
    onescol = nc.dram_tensor("onescol", [P, NKB * HPC], F16, kind="ExternalInput").ap()
    out = nc.dram_tensor("out", [T, D], F16, kind="ExternalOutput").ap()

    with tile.TileContext(nc) as tc:
        with tc.tile_pool(name="singles", bufs=1) as singles, \
             tc.tile_pool(name="rope16", bufs=3) as rope16, \
             tc.tile_pool(name="ptp", bufs=3) as ptp, \
             tc.tile_pool(name="obp", bufs=3) as obp, \
             tc.tile_pool(name="ssm", bufs=4) as ssm, \
             tc.tile_pool(name="bcp", bufs=4) as bcp, \
             tc.tile_pool(name="pspool", bufs=2, space="PSUM") as pspool:

            # ---- resident tiles ----
            xT_sb = singles.tile([P, DB, T], F16)
            wqT_sb = singles.tile([P, DB, CH], F16)
            wkT_sb = singles.tile([P, DB, CH], F16)
            wvT_sb = singles.tile([P, DB, CH], F16)
            woT_sb = singles.tile([P, CB, D], F16)
            cosS_sb = singles.tile([P, T], F16)
            sinS_sb = singles.tile([P, T], F16)
            ident_sb = singles.tile([P, P], F16)
            triB_sb = singles.tile([P, P], F16)
            QT_sb = singles.tile([P, CB, T], F16)
            KT_sb = singles.tile([P, CB, T], F16)
            attnT_sb = singles.tile([P, CB, T], F16)
            # V with a ones column per head: [kb, head, 65]
            vaug = singles.tile([P, NKB, HPC, HD + 1], F16)

            xTr = xT.rearrange("(o p) t -> p o t", p=P)

            # ---- resident loads, ordered so the pipeline can start ASAP ----
            nc.sync.dma_start(wqT_sb[:], wqT.rearrange("(o p) c -> p o c", p=P))
            for o in range(DB):
                nc.sync.dma_start(xT_sb[:, o, 0:NQ], xTr[:, o, 0:NQ])
            nc.sync.dma_start(wkT_sb[:], wkT.rearrange("(o p) c -> p o c", p=P))
            nc.sync.dma_start(cosS_sb[:], cosS[:])
            nc.sync.dma_start(sinS_sb[:], sinS[:])
            nc.sync.dma_start(ident_sb[:], ident[:])
            nc.sync.dma_start(triB_sb[:], triB[:])
            nc.sync.dma_start(wvT_sb[:], wvT.rearrange("(o p) c -> p o c", p=P))
            nc.sync.dma_start(
                vaug[:, :, :, HD:HD + 1],
                onescol.rearrange("p (a b) -> p a b", a=NKB)[:, :, :, None])
            for m in range(1, NT):
                ms = slice(m * NQ, (m + 1) * NQ)
                for o in range(DB):
                    nc.sync.dma_start(xT_sb[:, o, ms], xTr[:, o, ms])
            nc.sync.dma_start(woT_sb[:], woT.rearrange("(o p) c -> p o c", p=P))

            # ---------------- unit emitters (filler work) ----------------
            # Each unit is a list of closures; each closure emits ~1 PE matmul
            # (plus trailing cheap ops).  The attention kb-loop pops one step
            # per iteration so the PE always has independent fill work.

            def proj_unit(w_sb, dst_sb, cb, m, pname):
                """q/k projection of one 512-chunk + fp16 RoPE. 8 MM steps + tail."""
                cs = slice(m * NQ, (m + 1) * NQ)
                state = {}

                def mk_mm(o):
                    def step():
                        if o == 0:
                            state['ps'] = pspool.tile(
                                [P, NQ], F32, tag="b1", bufs=4,
                                name=f"prj_{pname}_{cb}_{m}")
                        nc.tensor.matmul(
                            state['ps'][:],
                            w_sb[:, o, cb * P:(cb + 1) * P],
                            xT_sb[:, o, cs],
                            start=(o == 0), stop=(o == DB - 1))
                    return step

                def tail():
                    ps = state['ps']
                    qraw = rope16.tile([P, NQ], F16, tag="qraw",
                                       name=f"qr_{pname}_{cb}_{m}")
                    nc.vector.tensor_copy(qraw[:], ps[:])
                    # q' = q*cos + swap32(q)*sin  (sign folded into sin table)
                    nc.vector.tensor_tensor(dst_sb[:, cb, cs], qraw[:],
                                            cosS_sb[:, cs], MUL)
                    qswp = rope16.tile([P, NQ], F16, tag="qswp",
                                       name=f"qs_{pname}_{cb}_{m}")
                    for g in range(4):
                        src = (g ^ 1) * 32
                        dst = g * 32
                        nc.gpsimd.dma_start(qswp[dst:dst + 32, :],
                                            qraw[src:src + 32, :])
                    tmp = rope16.tile([P, NQ], F16, tag="tmp",
                                      name=f"tm_{pname}_{cb}_{m}")
                    nc.vector.tensor_tensor(tmp[:], qswp[:], sinS_sb[:, cs], MUL)
                    nc.vector.tensor_tensor(dst_sb[:, cb, cs],
                                            dst_sb[:, cb, cs], tmp[:], ADD)

                return [mk_mm(o) for o in range(DB)] + [tail]

            def vproj_unit(i):
                """V projection of one 128-row t-block: 8 MM steps + cast tail."""
                state = {}

                def mk_mm(o):
                    def step():
                        if o == 0:
                            state['ps'] = pspool.tile(
                                [P, NQ], F32, tag="b1", bufs=4, name=f"v_{i}")
                        nc.tensor.matmul(
                            state['ps'][:, :CH],
                            xT_sb[:, o, i * P:(i + 1) * P],
                            wvT_sb[:, o, :],
                            start=(o == 0), stop=(o == DB - 1))
                    return step

                def tail():
                    nc.vector.tensor_copy(
                        vaug[:, i, :, 0:HD],
                        state['ps'][:, :CH].rearrange("p (h d) -> p h d", h=HPC))

                return [mk_mm(o) for o in range(DB)] + [tail]

            def wo_unit(i, j):
                """One [128,512] tile of the output projection: 2 MMs + DMA out."""
                state = {}

                def mk_mm(cb):
                    def step():
                        if cb == 0:
                            state['ps'] = pspool.tile(
                                [P, NQ], F32, tag="b1", bufs=4, name=f"o_{i}_{j}")
                        nc.tensor.matmul(
                            state['ps'][:],
                            attnT_sb[:, cb, i * P:(i + 1) * P],
                            woT_sb[:, cb, j * NQ:(j + 1) * NQ],
                            start=(cb == 0), stop=(cb == CB - 1))
                    return step

                def tail():
                    ob = obp.tile([P, NQ], F16, tag="ob", name=f"ob_{i}_{j}")
                    nc.vector.tensor_copy(ob[:], state['ps'][:])
                    nc.sync.dma_start(
                        out[i * P:(i + 1) * P, j * NQ:(j + 1) * NQ], ob[:])

                return [mk_mm(cb) for cb in range(CB)] + [tail]

            class StepQueue:
                def __init__(self):
                    self.steps = []

                def add_units(self, units):
                    for u in units:
                        self.steps.extend(u)

                def pop(self, n=1):
                    for _ in range(n):
                        if self.steps:
                            self.steps.pop(0)()

                def flush(self):
                    while self.steps:
                        self.steps.pop(0)()

            def kb_list(qc):
                return list(range(min(NKB, (qc + 1) * (NQ // P)))) if causal \
                    else list(range(NKB))

            # ---------------- attention ----------------
            def attention(hp, qc, fq):
                kbs = kb_list(qc)
                q0 = qc * NQ
                otps = [pspool.tile([HD + 1, NQ], F32, tag="b1", bufs=4,
                                    name=f"ot_{hp}_{qc}_{i}")
                        for i in range(2)]

                def finish(kb, stp2, qsl):
                    pt = ptp.tile([P, 2, NQ], F16, tag="pt",
                                  name=f"pt_{hp}_{qc}_{kb}")
                    sflat = stp2.rearrange("p a b -> p (a b)")
                    pflat = pt.rearrange("p a b -> p (a b)")
                    # one exp covers both halves; the uncomputed middle columns
                    # of diagonal blocks are never read downstream
                    nc.scalar.activation(pflat[:, qsl:2 * NQ],
                                         sflat[:, qsl:2 * NQ],
                                         AF.Exp, scale=float(HD) ** -0.5)
                    for half in range(2):
                        h = hp * 2 + half
                        nc.tensor.matmul(
                            otps[half][:, qsl:NQ],
                            vaug[:, kb, h, :],
                            pt[:, half, qsl:NQ],
                            start=(kb == kbs[0]), stop=(kb == kbs[-1]))

                pend = None
                for kb in kbs:
                    qsl = max(0, kb * P - q0) if causal else 0
                    diag = causal and kb * P >= q0
                    stp2 = pspool.tile([P, 2, NQ], F32, tag="st",
                                       name=f"st_{hp}_{qc}_{kb}")
                    for half in range(2):
                        hb = half * HD
                        nc.tensor.matmul(
                            stp2[:, half, qsl:NQ],
                            KT_sb[hb:hb + HD, hp, kb * P:(kb + 1) * P],
                            QT_sb[hb:hb + HD, hp, q0 + qsl:q0 + NQ],
                            start=True, stop=not diag)
                        if diag:
                            # causal mask: add -30000 strictly below the
                            # diagonal so exp underflows those to zero
                            nc.tensor.matmul(
                                stp2[:, half, qsl:qsl + P],
                                ident_sb[:],
                                triB_sb[:],
                                start=False, stop=True)
                    fq.pop(1)
                    if pend is not None:
                        finish(*pend)
                    pend = (kb, stp2, qsl)
                finish(*pend)
                return otps

            def normalize(hp, qc, otps):
                # reciprocal of the sums row + partition-broadcast + multiply,
                # reading the unnormalized output straight from PSUM
                for half in range(2):
                    otp = otps[half]
                    rs = ssm.tile([1, NQ], F32, tag="rs",
                                  name=f"rs_{hp}_{qc}_{half}")
                    nc.vector.reciprocal(rs[:], otp[HD:HD + 1, :])
                    bc = bcp.tile([HD, NQ], F32, tag="bc",
                                  name=f"bc_{hp}_{qc}_{half}")
                    nc.gpsimd.partition_broadcast(bc[:], rs[:])
                    nc.vector.tensor_tensor(
                        attnT_sb[half * HD:(half + 1) * HD, hp,
                                 qc * NQ:(qc + 1) * NQ],
                        otp[0:HD, :], bc[:], MUL)

            # ---------------- emission schedule ----------------
            fq = StepQueue()

            # prologue: everything attention(0,0) depends on, emitted densely
            fq.add_units([proj_unit(wqT_sb, QT_sb, 0, 0, "q"),
                          proj_unit(wkT_sb, KT_sb, 0, 0, "k")])
            fq.add_units([vproj_unit(i) for i in range(4)])
            fq.flush()

            prev_ots = {}
            for qc in range(NT):
                # fillers for att(0,qc): cb1 projections of this chunk (needed
                # by att(1,qc)) and the wo tiles of the previous chunk
                fq.add_units([proj_unit(wqT_sb, QT_sb, 1, qc, "q"),
                              proj_unit(wkT_sb, KT_sb, 1, qc, "k")])
                if qc > 0:
                    fq.add_units([wo_unit(i, j)
                                  for i in range(4 * (qc - 1), 4 * (qc - 1) + 2)
                                  for j in range(D // NQ)])
                ots0 = attention(0, qc, fq)
                fq.flush()
                normalize(0, qc, ots0)

                # fillers for att(1,qc): next chunk's cb0 projections + V rows
                if qc + 1 < NT:
                    fq.add_units([proj_unit(wqT_sb, QT_sb, 0, qc + 1, "q"),
                                  proj_unit(wkT_sb, KT_sb, 0, qc + 1, "k")])
                    fq.add_units([vproj_unit(i)
                                  for i in range(4 * (qc + 1), 4 * (qc + 2))])
                if qc > 0:
                    fq.add_units([wo_unit(i, j)
                                  for i in range(4 * (qc - 1) + 2, 4 * qc)
                                  for j in range(D // NQ)])
                ots1 = attention(1, qc, fq)
                fq.flush()
                normalize(1, qc, ots1)

            # tail: the last chunk's wo tiles
            fq.add_units([wo_unit(i, j)
                          for i in range(4 * (NT - 1), NKB)
                          for j in range(D // NQ)])
            fq.flush()

    nc.compile()
    return nc


def _get_program(causal: bool):
    key = ("causal" if causal else "full")
    if key not in _prog_cache:
        _prog_cache[key] = _build_program(causal)
    return _prog_cache[key]


def _mask_kind(mask):
    m = np.asarray(mask)
    if m.ndim == 4:
        m = m[0, 0]
    if (m != 0).all():
        return False  # full attention
    trilm = np.tril(np.ones((m.shape[0], m.shape[1]), dtype=m.dtype))
    if np.array_equal(m, trilm):
        return True
    raise NotImplementedError("mask is neither all-ones nor causal tril")


def _make_in_maps(x, cos, sin, wq, wk, wv, wo):
    x = np.asarray(x, dtype=np.float32)
    cos = np.asarray(cos, dtype=np.float32)
    sin = np.asarray(sin, dtype=np.float32)
    wq = np.asarray(wq, dtype=np.float32)
    wk = np.asarray(wk, dtype=np.float32)
    wv = np.asarray(wv, dtype=np.float32)
    wo = np.asarray(wo, dtype=np.float32)

    # RoPE tables in transposed head-pair layout [128ch, T].
    # cosS[c, t] = cos[t, c % 64]; sinS flips sign on the low half of each head
    # (rotate_half's minus), matching qswp[p] = q[p^32] on the device.
    ci = np.arange(P) % HD
    cosS = np.ascontiguousarray(cos[:T, ci].T.astype(np.float16))   # [128, T]
    sgn = np.where((np.arange(P) % HD) < (HD // 2), -1.0, 1.0).astype(np.float32)
    sinSm = np.ascontiguousarray(
        (sin[:T, ci].T * sgn[:, None]).astype(np.float16))          # [128, T]
    identm = np.eye(P, dtype=np.float16)
    triBm = np.ascontiguousarray(
        (np.tril(np.ones((P, P), np.float32), -1) * -30000.0).astype(np.float16))
    ones = np.ones((P, (T // P) * HPC), dtype=np.float16)

    in_maps = []
    for core in range(NCORES):
        b = core // GROUPS
        g = core % GROUPS
        c0 = g * CH
        in_maps.append({
            "xT": np.ascontiguousarray(x[b].T.astype(np.float16)),          # [D, T]
            "wqT": np.ascontiguousarray(wq[c0:c0 + CH, :].T.astype(np.float16)),
            "wkT": np.ascontiguousarray(wk[c0:c0 + CH, :].T.astype(np.float16)),
            "wvT": np.ascontiguousarray(wv[c0:c0 + CH, :].T.astype(np.float16)),
            "woT": np.ascontiguousarray(wo[:, c0:c0 + CH].T.astype(np.float16)),
            "cosS": cosS,
            "sinS": sinSm,
            "ident": identm,
            "triB": triBm,
            "onescol": ones,
        })
    return in_maps


def _run(inputs, trace=False):
    from concourse import bass_utils
    causal = _mask_kind(inputs["mask"])
    nc = _get_program(causal)
    in_maps = _make_in_maps(
        inputs["x"], inputs["cos"], inputs["sin"],
        inputs["wq"], inputs["wk"], inputs["wv"], inputs["wo"])
    if trace:
        _install_ntff_shim()
    res = bass_utils.run_bass_kernel_spmd(
        nc, in_maps, core_ids=list(range(NCORES)), trace=trace)
    outs = [r["out"] for r in res.results]
    full = np.empty((B, T, D), dtype=np.float32)
    for b in range(B):
        full[b] = outs[b * GROUPS].astype(np.float32)
        for g in range(1, GROUPS):
            full[b] += outs[b * GROUPS + g].astype(np.float32)
    return full, res


def kernel(**inputs):
    full, _ = _run(inputs, trace=False)
    return full


def kernel_profiled(**inputs):
    """Like kernel() but with NTFF tracing; returns (out, BassKernelResults)."""
    return _run(inputs, trace=True)


# revision 23
# speedup vs baseline: 1.1520x; 1.1520x over previous
"""Trainium2 Bass kernel for nn_Attention_47682726920277.

Causal multi-head attention with RoPE:
  q/k/v = x @ w{q,k,v}.T ; RoPE(q, k) ; att = softmax(mask(q k^T / 8)) ; out = (att v) @ wo.T
Shapes: x [2, 2048, 1024], 16 heads of dim 64, fp32.

Sharding (8 cores): data-parallel over batch (2) x tensor-parallel over heads (4 per
core). Each core computes its 4 heads' attention and a partial out via its wo row
block; the final all-reduce is the host-side sum of the 4 partials per batch.

Per-core design (v2 — software-pipelined):
  - Q,K produced transposed (QT/KT [256ch, T] fp16) so scores come out as S^T [k, q];
    V is augmented with a ones column so row 64 of the PV accumulator is the softmax
    denominator.  Exp runs on the scalar engine with the 1/8 scale fused; the scalar
    engine runs *only* exp (one activation table, loaded once).
  - RoPE runs at the DVE 2x fp16 rate: one cast from PSUM, a partition-XOR-32 swap
    done by 4 small SBUF->SBUF DMAs, then 3 full-width fp16 multiplies/adds.
  - Normalization: DVE reciprocal of the sums row, gpsimd partition-broadcast, one
    DVE multiply into attnT.  No DRAM roundtrip, no Ln on the scalar engine.
  - PSUM drains via direct DMA (PSUM->SBUF staging, PSUM->DRAM for the wo output)
    instead of vector-engine copies.
  - Emission interleaves the two head-pairs' attention per q-chunk and feeds the
    tensor engine projection/wo matmuls as fillers *inside* the attention kb-loop,
    so the PE never idles waiting on exp and the HAM clock gate stays at 2.4 GHz.
"""
import sys
import types
import numpy as np

B = 2
T = 2048
D = 1024
H = 16
HD = 64
NCORES = 8
GROUPS = NCORES // B          # head-groups per batch
HPC = H // GROUPS             # heads per core = 4
CH = HPC * HD                 # channels per core = 256
NQ = 512                      # PSUM bank width (fp32)
P = 128

_prog_cache = {}


def _install_ntff_shim():
    """The agent image's antenv lacks axon_hooks; inject it so trace=True works."""
    try:
        import antenv.axon_hooks  # noqa: F401
        return
    except ImportError:
        pass
    try:
        import trn_agent_boot.trn_boot as tb
        hook = tb._ntff_profile_via_ctypes('/opt/axon/libaxon_pjrt.so')
        if hook is None:
            return
        mod = types.ModuleType('antenv.axon_hooks')
        mod.get_axon_ntff_profile_hook = lambda: hook
        mod.set_axon_ntff_profile_hook = lambda h: None
        sys.modules['antenv.axon_hooks'] = mod
        import antenv
        antenv.axon_hooks = mod
    except Exception:
        pass


def _build_program(causal: bool):
    import concourse.bass as bass
    from concourse import bacc
    import concourse.tile as tile
    from concourse import mybir

    F32 = mybir.dt.float32
    F16 = mybir.dt.float16
    AF = mybir.ActivationFunctionType
    MUL = mybir.AluOpType.mult
    ADD = mybir.AluOpType.add

    NT = T // NQ          # proj/attention q-chunks (4)
    NKB = T // P          # k-blocks (16)
    DB = D // P           # d-blocks (8)
    CB = CH // P          # channel blocks = head-pair blocks (2)

    nc = bacc.Bacc("TRN2", target_bir_lowering=False, debug=False)

    xT = nc.dram_tensor("xT", [D, T], F16, kind="ExternalInput").ap()
    wqT = nc.dram_tensor("wqT", [D, CH], F16, kind="ExternalInput").ap()
    wkT = nc.dram_tensor("wkT", [D, CH], F16, kind="ExternalInput").ap()
    wvT = nc.dram_tensor("wvT", [D, CH], F16, kind="ExternalInput").ap()
    woT = nc.dram_tensor("woT", [CH, D], F16, kind="ExternalInput").ap()
    cosS = nc.dram_tensor("cosS", [P, T], F16, kind="ExternalInput").ap()
    sinS = nc.dram_tensor("sinS", [P, T], F16, kind="ExternalInput").ap()
    ident = nc.dram_tensor("ident", [P, P], F16, kind="ExternalInput").ap()
    triB = nc.dram_tensor("triB", [P, P], F16, kind="ExternalInput").ap()
    onescol = nc.dram_tensor("onescol", [P, NKB * HPC], F16, kind="ExternalInput").ap()
    out = nc.dram_tensor("out", [T, D], F16, kind="ExternalOutput").ap()

    with tile.TileContext(nc) as tc:
        with tc.tile_pool(name="singles", bufs=1) as singles, \
             tc.tile_pool(name="rope16", bufs=3) as rope16, \
             tc.tile_pool(name="ptp", bufs=3) as ptp, \
             tc.tile_pool(name="obp", bufs=3) as obp, \
             tc.tile_pool(name="ssm", bufs=2) as ssm, \
             tc.tile_pool(name="bcp", bufs=4) as bcp, \
             tc.tile_pool(name="dramp", bufs=1, space="DRAM") as dramp, \
             tc.tile_pool(name="pspool", bufs=2, space="PSUM") as pspool:

            # ---- resident tiles ----
            xT_sb = singles.tile([P, DB, T], F16)
            wqT_sb = singles.tile([P, DB, CH], F16)
            wkT_sb = singles.tile([P, DB, CH], F16)
            wvT_sb = singles.tile([P, DB, CH], F16)
            woT_sb = singles.tile([P, CB, D], F16)
            cosS_sb = singles.tile([P, T], F16)
            sinS_sb = singles.tile([P, T], F16)
            ident_sb = singles.tile([P, P], F16)
            triB_sb = singles.tile([P, P], F16)
            QT_sb = singles.tile([P, CB, T], F16)
            KT_sb = singles.tile([P, CB, T], F16)
            attnT_sb = singles.tile([P, CB, T], F16)
            # V with a ones column per head: [kb, head, 65]
            vaug = singles.tile([P, NKB, HPC, HD + 1], F16)
            # staged unnormalized attention outputs [65, hp, qc*2+half, q]
            otsb = singles.tile([HD + 1, CB, 2 * NT, NQ], F32)
            recd = dramp.tile([NT, 4, NQ], F32)

            xTr = xT.rearrange("(o p) t -> p o t", p=P)

            # ---- resident loads, ordered so the pipeline can start ASAP ----
            nc.sync.dma_start(wqT_sb[:], wqT.rearrange("(o p) c -> p o c", p=P))
            for o in range(DB):
                nc.sync.dma_start(xT_sb[:, o, 0:NQ], xTr[:, o, 0:NQ])
            nc.sync.dma_start(wkT_sb[:], wkT.rearrange("(o p) c -> p o c", p=P))
            nc.sync.dma_start(cosS_sb[:], cosS[:])
            nc.sync.dma_start(sinS_sb[:], sinS[:])
            nc.sync.dma_start(ident_sb[:], ident[:])
            nc.sync.dma_start(triB_sb[:], triB[:])
            nc.sync.dma_start(wvT_sb[:], wvT.rearrange("(o p) c -> p o c", p=P))
            nc.sync.dma_start(
                vaug[:, :, :, HD:HD + 1],
                onescol.rearrange("p (a b) -> p a b", a=NKB)[:, :, :, None])
            for o in range(DB):
                nc.sync.dma_start(xT_sb[:, o, NQ:T], xTr[:, o, NQ:T])
            nc.sync.dma_start(woT_sb[:], woT.rearrange("(o p) c -> p o c", p=P))

            # ---------------- unit emitters (filler work) ----------------
            # Each unit is a list of closures; each closure emits ~1 PE matmul
            # (plus trailing cheap ops).  The attention kb-loop pops one step
            # per iteration so the PE always has independent fill work.

            def proj_unit(w_sb, dst_sb, cb, m, pname):
                """q/k projection of one 512-chunk + fp16 RoPE. 8 MM steps + tail."""
                cs = slice(m * NQ, (m + 1) * NQ)
                state = {}

                def mk_mm(o):
                    def step():
                        if o == 0:
                            state['ps'] = pspool.tile(
                                [P, NQ], F32, tag="b1", bufs=4,
                                name=f"prj_{pname}_{cb}_{m}")
                        nc.tensor.matmul(
                            state['ps'][:],
                            w_sb[:, o, cb * P:(cb + 1) * P],
                            xT_sb[:, o, cs],
                            start=(o == 0), stop=(o == DB - 1))
                    return step

                def tail():
                    ps = state['ps']
                    qraw = rope16.tile([P, NQ], F16, tag="qraw",
                                       name=f"qr_{pname}_{cb}_{m}")
                    nc.scalar.activation(qraw[:], ps[:], AF.Copy)
                    # q' = q*cos + swap32(q)*sin  (sign folded into sin table)
                    nc.vector.tensor_tensor(dst_sb[:, cb, cs], qraw[:],
                                            cosS_sb[:, cs], MUL)
                    qswp = rope16.tile([P, NQ], F16, tag="qswp",
                                       name=f"qs_{pname}_{cb}_{m}")
                    for g in range(4):
                        src = (g ^ 1) * 32
                        dst = g * 32
                        nc.gpsimd.dma_start(qswp[dst:dst + 32, :],
                                            qraw[src:src + 32, :])
                    tmp = rope16.tile([P, NQ], F16, tag="tmp",
                                      name=f"tm_{pname}_{cb}_{m}")
                    nc.vector.tensor_tensor(tmp[:], qswp[:], sinS_sb[:, cs], MUL)
                    nc.vector.tensor_tensor(dst_sb[:, cb, cs],
                                            dst_sb[:, cb, cs], tmp[:], ADD)

                return [mk_mm(o) for o in range(DB)] + [tail]

            def vproj_unit(i):
                """V projection of one 128-row t-block: 8 MM steps + cast tail."""
                state = {}

                def mk_mm(o):
                    def step():
                        if o == 0:
                            state['ps'] = pspool.tile(
                                [P, NQ], F32, tag="b1", bufs=4, name=f"v_{i}")
                        nc.tensor.matmul(
                            state['ps'][:, :CH],
                            xT_sb[:, o, i * P:(i + 1) * P],
                            wvT_sb[:, o, :],
                            start=(o == 0), stop=(o == DB - 1))
                    return step

                def tail():
                    nc.vector.tensor_copy(
                        vaug[:, i, :, 0:HD],
                        state['ps'][:, :CH].rearrange("p (h d) -> p h d", h=HPC))

                return [mk_mm(o) for o in range(DB)] + [tail]

            def wo_unit(i, j):
                """One [128,512] tile of the output projection: 2 MMs + DMA out."""
                state = {}

                def mk_mm(cb):
                    def step():
                        if cb == 0:
                            state['ps'] = pspool.tile(
                                [P, NQ], F32, tag="b1", bufs=4, name=f"o_{i}_{j}")
                        nc.tensor.matmul(
                            state['ps'][:],
                            attnT_sb[:, cb, i * P:(i + 1) * P],
                            woT_sb[:, cb, j * NQ:(j + 1) * NQ],
                            start=(cb == 0), stop=(cb == CB - 1))
                    return step

                def tail():
                    ob = obp.tile([P, NQ], F16, tag="ob", name=f"ob_{i}_{j}")
                    nc.vector.tensor_copy(ob[:], state['ps'][:])
                    nc.gpsimd.dma_start(
                        out[i * P:(i + 1) * P, j * NQ:(j + 1) * NQ], ob[:])

                return [mk_mm(cb) for cb in range(CB)] + [tail]

            class StepQueue:
                def __init__(self):
                    self.steps = []

                def add_units(self, units):
                    for u in units:
                        self.steps.extend(u)

                def pop(self, n=1):
                    for _ in range(n):
                        if self.steps:
                            self.steps.pop(0)()

                def flush(self):
                    while self.steps:
                        self.steps.pop(0)()

            def kb_list(qc):
                return list(range(min(NKB, (qc + 1) * (NQ // P)))) if causal \
                    else list(range(NKB))

            # ---------------- attention ----------------
            def attention(hp, qc, fq, pre_iter=None):
                kbs = kb_list(qc)
                q0 = qc * NQ
                otps = [pspool.tile([HD + 1, NQ], F32, tag="b1", bufs=4,
                                    name=f"ot_{hp}_{qc}_{i}")
                        for i in range(2)]

                def finish(kb, stp2, qsl):
                    pt = ptp.tile([P, 2, NQ], F16, tag="pt",
                                  name=f"pt_{hp}_{qc}_{kb}")
                    sflat = stp2.rearrange("p a b -> p (a b)")
                    pflat = pt.rearrange("p a b -> p (a b)")
                    # one exp covers both halves; the uncomputed middle columns
                    # of diagonal blocks are never read downstream
                    nc.scalar.activation(pflat[:, qsl:2 * NQ],
                                         sflat[:, qsl:2 * NQ],
                                         AF.Exp, scale=float(HD) ** -0.5)
                    for half in range(2):
                        h = hp * 2 + half
                        nc.tensor.matmul(
                            otps[half][:, qsl:NQ],
                            vaug[:, kb, h, :],
                            pt[:, half, qsl:NQ],
                            start=(kb == kbs[0]), stop=(kb == kbs[-1]))

                pend = None
                for kb in kbs:
                    if pre_iter is not None:
                        pre_iter(kb)
                    qsl = max(0, kb * P - q0) if causal else 0
                    diag = causal and kb * P >= q0
                    stp2 = pspool.tile([P, 2, NQ], F32, tag="st",
                                       name=f"st_{hp}_{qc}_{kb}")
                    # both halves' score matmuls back-to-back so the two
                    # K=64 row-groups run concurrently; masks after
                    for half in range(2):
                        hb = half * HD
                        nc.tensor.matmul(
                            stp2[:, half, qsl:NQ],
                            KT_sb[hb:hb + HD, hp, kb * P:(kb + 1) * P],
                            QT_sb[hb:hb + HD, hp, q0 + qsl:q0 + NQ],
                            start=True, stop=not diag)
                    if diag:
                        # causal mask: add -30000 strictly below the
                        # diagonal so exp underflows those to zero
                        for half in range(2):
                            nc.tensor.matmul(
                                stp2[:, half, qsl:qsl + P],
                                ident_sb[:],
                                triB_sb[:],
                                start=False, stop=True)
                    fq.pop(1)
                    if pend is not None:
                        finish(*pend)
                    pend = (kb, stp2, qsl)
                finish(*pend)
                return otps

            def stage(hp, qc, otps):
                # drain PSUM into the staged SBUF buffer (frees the banks)
                for half in range(2):
                    nc.vector.tensor_copy(otsb[:, hp, qc * 2 + half, :],
                                          otps[half][:])

            def normalize(qc):
                # pack the 4 sums rows onto 4 partitions, one batched
                # reciprocal (DVE reciprocal cost is per-column), broadcast
                # back across partitions via a DRAM row re-read, multiply
                sums4 = ssm.tile([4, NQ], F32, tag="s4", name=f"s4_{qc}")
                for r in range(4):
                    hp, half = divmod(r, 2)
                    nc.sync.dma_start(sums4[r:r + 1, :],
                                      otsb[HD:HD + 1, hp, qc * 2 + half, :])
                rec4 = ssm.tile([4, NQ], F32, tag="r4", name=f"r4_{qc}")
                nc.vector.reciprocal(rec4[:], sums4[:])
                nc.sync.dma_start(recd[qc], rec4[:])
                for r in range(4):
                    hp, half = divmod(r, 2)
                    bc = bcp.tile([HD, NQ], F32, tag="bc",
                                  name=f"bc_{qc}_{r}")
                    row = recd[qc, r]
                    src = bass.AP(tensor=row.tensor, offset=row.offset,
                                  ap=[[0, HD]] + list(row.ap))
                    nc.gpsimd.dma_start(bc[:], src)
                    nc.vector.tensor_tensor(
                        attnT_sb[half * HD:(half + 1) * HD, hp,
                                 qc * NQ:(qc + 1) * NQ],
                        otsb[0:HD, hp, qc * 2 + half, :], bc[:], MUL)

            # ---------------- emission schedule ----------------
            fq = StepQueue()

            # prologue: q/k cb0 m0 projections (att(0,0)'s V rows are emitted
            # inside the kb loop so exp starts as early as possible)
            fq.add_units([proj_unit(wqT_sb, QT_sb, 0, 0, "q"),
                          proj_unit(wkT_sb, KT_sb, 0, 0, "k")])
            fq.flush()

            for qc in range(NT):
                # fillers for att(0,qc): cb1 projections of this chunk (needed
                # by att(1,qc)), then the wo tiles of the previous chunk
                fq.add_units([proj_unit(wqT_sb, QT_sb, 1, qc, "q"),
                              proj_unit(wkT_sb, KT_sb, 1, qc, "k")])
                if qc > 0:
                    fq.add_units([wo_unit(i, j)
                                  for i in range(4 * (qc - 1), 4 * qc)
                                  for j in range(D // NQ)])

                # att(0,0): V row kb must be resident before finish(kb) reads
                # it, so emit each vproj unit inline at the top of iteration kb
                def pre0(kb):
                    for step in vproj_unit(kb):
                        step()
                ots0 = attention(0, qc, fq, pre_iter=pre0 if qc == 0 else None)
                fq.flush()
                stage(0, qc, ots0)

                # fillers for att(1,qc): next chunk's cb0 projections + V rows
                if qc + 1 < NT:
                    fq.add_units([proj_unit(wqT_sb, QT_sb, 0, qc + 1, "q"),
                                  proj_unit(wkT_sb, KT_sb, 0, qc + 1, "k")])
                    fq.add_units([vproj_unit(i)
                                  for i in range(4 * (qc + 1), 4 * (qc + 2))])
                ots1 = attention(1, qc, fq)
                fq.flush()
                stage(1, qc, ots1)
                normalize(qc)

            # tail: the last chunk's wo tiles
            fq.add_units([wo_unit(i, j)
                          for i in range(4 * (NT - 1), NKB)
                          for j in range(D // NQ)])
            fq.flush()

    nc.compile()
    return nc


def _get_program(causal: bool):
    key = ("causal" if causal else "full")
    if key not in _prog_cache:
        _prog_cache[key] = _build_program(causal)
    return _prog_cache[key]


def _mask_kind(mask):
    m = np.asarray(mask)
    if m.ndim == 4:
        m = m[0, 0]
    if (m != 0).all():
        return False  # full attention
    trilm = np.tril(np.ones((m.shape[0], m.shape[1]), dtype=m.dtype))
    if np.array_equal(m, trilm):
        return True
    raise NotImplementedError("mask is neither all-ones nor causal tril")


def _make_in_maps(x, cos, sin, wq, wk, wv, wo):
    x = np.asarray(x, dtype=np.float32)
    cos = np.asarray(cos, dtype=np.float32)
    sin = np.asarray(sin, dtype=np.float32)
    wq = np.asarray(wq, dtype=np.float32)
    wk = np.asarray(wk, dtype=np.float32)
    wv = np.asarray(wv, dtype=np.float32)
    wo = np.asarray(wo, dtype=np.float32)

    # RoPE tables in transposed head-pair layout [128ch, T].
    # cosS[c, t] = cos[t, c % 64]; sinS flips sign on the low half of each head
    # (rotate_half's minus), matching qswp[p] = q[p^32] on the device.
    ci = np.arange(P) % HD
    cosS = np.ascontiguousarray(cos[:T, ci].T.astype(np.float16))   # [128, T]
    sgn = np.where((np.arange(P) % HD) < (HD // 2), -1.0, 1.0).astype(np.float32)
    sinSm = np.ascontiguousarray(
        (sin[:T, ci].T * sgn[:, None]).astype(np.float16))          # [128, T]
    identm = np.eye(P, dtype=np.float16)
    triBm = np.ascontiguousarray(
        (np.tril(np.ones((P, P), np.float32), -1) * -30000.0).astype(np.float16))
    ones = np.ones((P, (T // P) * HPC), dtype=np.float16)

    in_maps = []
    for core in range(NCORES):
        b = core // GROUPS
        g = core % GROUPS
        c0 = g * CH
        in_maps.append({
            "xT": np.ascontiguousarray(x[b].T.astype(np.float16)),          # [D, T]
            "wqT": np.ascontiguousarray(wq[c0:c0 + CH, :].T.astype(np.float16)),
            "wkT": np.ascontiguousarray(wk[c0:c0 + CH, :].T.astype(np.float16)),
            "wvT": np.ascontiguousarray(wv[c0:c0 + CH, :].T.astype(np.float16)),
            "woT": np.ascontiguousarray(wo[:, c0:c0 + CH].T.astype(np.float16)),
            "cosS": cosS,
            "sinS": sinSm,
            "ident": identm,
            "triB": triBm,
            "onescol": ones,
        })
    return in_maps


def _run(inputs, trace=False):
    from concourse import bass_utils
    causal = _mask_kind(inputs["mask"])
    nc = _get_program(causal)
    in_maps = _make_in_maps(
        inputs["x"], inputs["cos"], inputs["sin"],
        inputs["wq"], inputs["wk"], inputs["wv"], inputs["wo"])
    if trace:
        _install_ntff_shim()
    res = bass_utils.run_bass_kernel_spmd(
        nc, in_maps, core_ids=list(range(NCORES)), trace=trace)
    outs = [r["out"] for r in res.results]
    full = np.empty((B, T, D), dtype=np.float32)
    for b in range(B):
        full[b] = outs[b * GROUPS].astype(np.float32)
        for g in range(1, GROUPS):
            full[b] += outs[b * GROUPS + g].astype(np.float32)
    return full, res


def kernel(**inputs):
    full, _ = _run(inputs, trace=False)
    return full


def kernel_profiled(**inputs):
    """Like kernel() but with NTFF tracing; returns (out, BassKernelResults)."""
    return _run(inputs, trace=True)
